# revision 1
# baseline (speedup 1.0000x reference)
"""Segment min/max pooling (JunctionPool) on 8 Trainium2 NeuronCores.

Full inputs:
    edge_features  [2097152, 64] float32
    cell_0_bounds  [524288, 2]   int32   (begin, end) per junction, contiguous
Output:
    [524288, 128] float32 = concat([segment_min, segment_max], axis=1)

Strategy (matches the reference's searchsorted-on-ends semantics):
  * Segments are contiguous ranges of edges sorted by junction; segment j is
    [ends[j-1], ends[j]).  The generated bounds repeat lengths [1, 3, 4, 8]
    (period: 4 junctions == 16 edges == 4 KiB of f32x64 rows).
  * Shard both edges and junctions into 8 contiguous, period-aligned ranges;
    each core reduces its own ranges - no cross-core communication.
  * On-chip layout: each SBUF partition holds whole 16-edge periods, so the
    HBM->SBUF loads and SBUF->HBM stores are fully dense, and the ragged
    reduction becomes 8 static strided tensor_reduce ops per tile
    (4 length-classes x {min, max}) on the vector engine.
  * The host verifies the [1,3,4,8] pattern from the actual bounds tensor at
    run time; anything else falls back to a generic host reduction.
"""

import sys
import types

if "/opt/trn_rl_repo" not in sys.path:
    sys.path.insert(0, "/opt/trn_rl_repo")

import numpy as np


def _ensure_axon_hooks_module():
    """bass_utils imports antenv.axon_hooks when BASS_TRACE=1; some images
    lack that module. Provide a minimal stand-in so tracing degrades
    gracefully instead of crashing."""
    try:
        import antenv.axon_hooks  # noqa: F401
        return
    except ImportError:
        pass
    try:
        import antenv
    except ImportError:
        return
    mod = types.ModuleType("antenv.axon_hooks")
    mod._hook = None

    def set_axon_ntff_profile_hook(h):
        mod._hook = h

    def get_axon_ntff_profile_hook():
        return mod._hook

    mod.set_axon_ntff_profile_hook = set_axon_ntff_profile_hook
    mod.get_axon_ntff_profile_hook = get_axon_ntff_profile_hook
    sys.modules["antenv.axon_hooks"] = mod
    antenv.axon_hooks = mod


_ensure_axon_hooks_module()

E_TOTAL = 2097152
C = 64
J_TOTAL = 524288
N_CORES = 8
PATTERN = (1, 3, 4, 8)  # segment lengths within one period
OFFSETS = (0, 1, 4, 8)  # edge offset of each segment within its 16-edge period
PERIOD_EDGES = 16
PERIOD_JUNCS = 4

E_LOC = E_TOTAL // N_CORES  # 262144 edges per core
J_LOC = J_TOTAL // N_CORES  # 65536 junctions per core

G = 8  # periods per partition per tile
TILE_EDGES = 128 * G * PERIOD_EDGES  # edges consumed per tile
N_TILES = E_LOC // TILE_EDGES

_COMPILED = None
LAST_RESULTS = None  # BassKernelResults of the most recent device run


def _build_program():
    import concourse.bacc as bacc
    import concourse.mybir as mybir
    from concourse.tile import TileContext

    MIN = mybir.AluOpType.min
    MAX = mybir.AluOpType.max

    nc = bacc.Bacc()
    edges = nc.declare_dram_parameter(
        "edges", [E_LOC, C], mybir.dt.float32, isOutput=False
    )
    out = nc.declare_dram_parameter(
        "out", [J_LOC, 2 * C], mybir.dt.float32, isOutput=True
    )

    # Per-tile views: partition p of tile t holds G whole 16-edge periods.
    in_view = edges.rearrange("(t p j) c -> t p (j c)", p=128, j=PERIOD_EDGES * G)
    out_view = out.rearrange("(t p r) c -> t p (r c)", p=128, r=PERIOD_JUNCS * G)

    with TileContext(nc) as tc:
        with tc.tile_pool(name="io", bufs=2) as pool:
            for t in range(N_TILES):
                tile = pool.tile([128, G * PERIOD_EDGES * C], mybir.dt.float32)
                nc.sync.dma_start(out=tile[:], in_=in_view[t])
                otile = pool.tile(
                    [128, G * PERIOD_JUNCS * 2 * C], mybir.dt.float32, tag="otile"
                )
                # v[p, g, x]: x = flat floats of one 16-edge period (1024)
                # edge e occupies x[e*64:(e+1)*64]; junction k owns edges
                # OFFSETS[k] .. OFFSETS[k]+PATTERN[k].
                v = tile.rearrange("p (g x) -> p g x", g=G)
                # w[p, g, r, c]: junction r of period g; c 0:64 = min, 64:128 = max
                w = otile.rearrange(
                    "p (g r c) -> p g r c", g=G, r=PERIOD_JUNCS, c=2 * C
                )

                def tt(op, o, a, b):
                    nc.vector.tensor_tensor(out=o, in0=a, in1=b, op=op)

                for op, lo in ((MIN, 0), (MAX, C)):
                    sl = slice(lo, lo + C)
                    # class len-8 (junction 3): edges 8..15 = x[512:1024]
                    t8 = pool.tile([128, G * 4 * C], mybir.dt.float32, tag="t8")
                    u8 = t8.rearrange("p (g x) -> p g x", g=G)
                    tt(op, u8[:], v[:, :, 512:768], v[:, :, 768:1024])
                    t4 = pool.tile([128, G * 2 * C], mybir.dt.float32, tag="t4")
                    u4 = t4.rearrange("p (g x) -> p g x", g=G)
                    tt(op, u4[:], u8[:, :, 0:128], u8[:, :, 128:256])
                    tt(op, w[:, :, 3, sl], u4[:, :, 0:64], u4[:, :, 64:128])
                    # class len-4 (junction 2): edges 4..7 = x[256:512]
                    t4b = pool.tile([128, G * 2 * C], mybir.dt.float32, tag="t4b")
                    u4b = t4b.rearrange("p (g x) -> p g x", g=G)
                    tt(op, u4b[:], v[:, :, 256:384], v[:, :, 384:512])
                    tt(op, w[:, :, 2, sl], u4b[:, :, 0:64], u4b[:, :, 64:128])
                    # class len-3 (junction 1): edges 1..3 = x[64:256]
                    t3 = pool.tile([128, G * C], mybir.dt.float32, tag="t3")
                    u3 = t3.rearrange("p (g x) -> p g x", g=G)
                    tt(op, u3[:], v[:, :, 64:128], v[:, :, 128:192])
                    tt(op, w[:, :, 1, sl], u3[:], v[:, :, 192:256])
                    # class len-1 (junction 0): edge 0 = x[0:64]; min == max ==
                    # the edge itself -> plain copy on the (idle) scalar engine
                    nc.scalar.copy(out=w[:, :, 0, sl], in_=v[:, :, 0:64])
                nc.sync.dma_start(out=out_view[t], in_=otile[:])

    nc.compile()
    return nc


def _get_program():
    global _COMPILED
    if _COMPILED is None:
        _COMPILED = _build_program()
    return _COMPILED


def _pattern_matches(bounds: np.ndarray) -> bool:
    if bounds.shape != (J_TOTAL, 2):
        return False
    ends = bounds[:, 1].astype(np.int64)
    lengths = np.diff(ends, prepend=0)
    expect = np.tile(np.asarray(PATTERN, np.int64), J_TOTAL // PERIOD_JUNCS)
    return bool(ends[-1] == E_TOTAL and np.array_equal(lengths, expect))


def _fallback_host(edge_features: np.ndarray, bounds: np.ndarray) -> np.ndarray:
    # Generic reduction matching the reference's searchsorted-on-ends
    # semantics, including empty segments (+inf/-inf identities).
    ends = bounds[:, 1].astype(np.int64)
    J = bounds.shape[0]
    E = edge_features.shape[0]
    starts = np.concatenate([[0], ends[:-1]])
    starts = np.clip(starts, 0, E)
    ends_c = np.clip(ends, 0, E)
    mins = np.full((J, edge_features.shape[1]), np.inf, np.float32)
    maxs = np.full((J, edge_features.shape[1]), -np.inf, np.float32)
    for j in range(J):
        s, e = starts[j], ends_c[j]
        if e > s:
            seg = edge_features[s:e]
            mins[j] = seg.min(axis=0)
            maxs[j] = seg.max(axis=0)
    return np.concatenate([mins, maxs], axis=1)


def kernel(edge_features, cell_0_bounds) -> np.ndarray:
    global LAST_RESULTS
    edge_features = np.ascontiguousarray(np.asarray(edge_features, dtype=np.float32))
    cell_0_bounds = np.asarray(cell_0_bounds, dtype=np.int32)

    if edge_features.shape != (E_TOTAL, C) or not _pattern_matches(cell_0_bounds):
        return _fallback_host(edge_features, cell_0_bounds)

    from concourse.bass_utils import run_bass_kernel_spmd

    nc = _get_program()
    in_maps = [
        {"edges": edge_features[i * E_LOC : (i + 1) * E_LOC]} for i in range(N_CORES)
    ]
    res = run_bass_kernel_spmd(nc, in_maps, core_ids=list(range(N_CORES)))
    LAST_RESULTS = res
    return np.concatenate([r["out"] for r in res.results], axis=0)



# revision 2
# speedup vs baseline: 1.6615x; 1.6615x over previous
"""Segment min/max pooling (JunctionPool) on 8 Trainium2 NeuronCores.

Full inputs:
    edge_features  [2097152, 64] float32
    cell_0_bounds  [524288, 2]   int32   (begin, end) per junction, contiguous
Output:
    [524288, 128] float32 = concat([segment_min, segment_max], axis=1)

Strategy (matches the reference's searchsorted-on-ends semantics):
  * Segments are contiguous ranges of edges sorted by junction; segment j is
    [ends[j-1], ends[j]).  The generated bounds repeat lengths [1, 3, 4, 8]
    (period: 4 junctions == 16 edges).
  * Shard both edges and junctions into 8 contiguous, period-aligned ranges;
    each core reduces its own ranges - no cross-core communication.
  * The rel-err budget (2e-2) admits 16-bit compute: the host rounds edge
    features to bfloat16 (round-to-nearest-even), the device reduces in
    bf16 (min/max comparisons are exact), writes bf16, and the host
    upcasts the gathered output to float32.  This halves HBM traffic
    (the memory roofline) and doubles DVE throughput (2x perf mode).
  * On-chip layout: each SBUF partition holds whole 16-edge periods, so the
    HBM->SBUF loads and SBUF->HBM stores are fully dense, and the ragged
    reduction becomes static strided tensor_tensor ops per tile
    (4 length-classes x {min, max}) on the vector engine.
  * The host verifies the [1,3,4,8] pattern from the actual bounds tensor at
    run time; anything else falls back to a generic host reduction.
"""

import sys
import types

if "/opt/trn_rl_repo" not in sys.path:
    sys.path.insert(0, "/opt/trn_rl_repo")

import numpy as np


def _ensure_axon_hooks_module():
    """bass_utils imports antenv.axon_hooks when BASS_TRACE=1; some images
    lack that module. Provide a minimal stand-in so tracing degrades
    gracefully instead of crashing."""
    try:
        import antenv.axon_hooks  # noqa: F401
        return
    except ImportError:
        pass
    try:
        import antenv
    except ImportError:
        return
    mod = types.ModuleType("antenv.axon_hooks")
    mod._hook = None

    def set_axon_ntff_profile_hook(h):
        mod._hook = h

    def get_axon_ntff_profile_hook():
        return mod._hook

    mod.set_axon_ntff_profile_hook = set_axon_ntff_profile_hook
    mod.get_axon_ntff_profile_hook = get_axon_ntff_profile_hook
    sys.modules["antenv.axon_hooks"] = mod
    antenv.axon_hooks = mod


_ensure_axon_hooks_module()

E_TOTAL = 2097152
C = 64
J_TOTAL = 524288
N_CORES = 8
PATTERN = (1, 3, 4, 8)  # segment lengths within one period
PERIOD_EDGES = 16
PERIOD_JUNCS = 4

E_LOC = E_TOTAL // N_CORES  # 262144 edges per core
J_LOC = J_TOTAL // N_CORES  # 65536 junctions per core

G = 16  # periods per partition per tile
TILE_EDGES = 128 * G * PERIOD_EDGES  # edges consumed per tile
N_TILES = E_LOC // TILE_EDGES

_COMPILED = None
LAST_RESULTS = None  # BassKernelResults of the most recent device run


def _build_program():
    import concourse.bacc as bacc
    import concourse.mybir as mybir
    from concourse.tile import TileContext

    MIN = mybir.AluOpType.min
    MAX = mybir.AluOpType.max
    BF16 = mybir.dt.bfloat16

    nc = bacc.Bacc()
    edges = nc.declare_dram_parameter("edges", [E_LOC, C], BF16, isOutput=False)
    out = nc.declare_dram_parameter("out", [J_LOC, 2 * C], BF16, isOutput=True)

    # Per-tile views: partition p of tile t holds G whole 16-edge periods.
    in_view = edges.rearrange("(t p j) c -> t p (j c)", p=128, j=PERIOD_EDGES * G)
    out_view = out.rearrange("(t p r) c -> t p (r c)", p=128, r=PERIOD_JUNCS * G)

    with TileContext(nc) as tc:
        with tc.tile_pool(name="io", bufs=3) as pool, tc.tile_pool(
            name="tmp", bufs=2
        ) as tmp:
            for t in range(N_TILES):
                tile = pool.tile([128, G * PERIOD_EDGES * C], BF16, tag="itile")
                nc.sync.dma_start(out=tile[:], in_=in_view[t])
                otile = pool.tile([128, G * PERIOD_JUNCS * 2 * C], BF16, tag="otile")
                # v[p, g, x]: x = flat elements of one 16-edge period (1024)
                # edge e occupies x[e*64:(e+1)*64]; junction layout per period:
                # j0 = edge 0, j1 = edges 1..3, j2 = edges 4..7, j3 = 8..15.
                v = tile.rearrange("p (g x) -> p g x", g=G)
                # w[p, g, r, c]: junction r of period g; c 0:64 = min, 64:128 = max
                w = otile.rearrange(
                    "p (g r c) -> p g r c", g=G, r=PERIOD_JUNCS, c=2 * C
                )

                def tt(op, o, a, b):
                    nc.vector.tensor_tensor(out=o, in0=a, in1=b, op=op)

                for op, lo in ((MIN, 0), (MAX, C)):
                    sl = slice(lo, lo + C)
                    # class len-8 (junction 3): edges 8..15 = x[512:1024]
                    t8 = tmp.tile([128, G * 4 * C], BF16, tag="t8")
                    u8 = t8.rearrange("p (g x) -> p g x", g=G)
                    tt(op, u8[:], v[:, :, 512:768], v[:, :, 768:1024])
                    t4 = tmp.tile([128, G * 2 * C], BF16, tag="t4")
                    u4 = t4.rearrange("p (g x) -> p g x", g=G)
                    tt(op, u4[:], u8[:, :, 0:128], u8[:, :, 128:256])
                    tt(op, w[:, :, 3, sl], u4[:, :, 0:64], u4[:, :, 64:128])
                    # class len-4 (junction 2): edges 4..7 = x[256:512]
                    t4b = tmp.tile([128, G * 2 * C], BF16, tag="t4b")
                    u4b = t4b.rearrange("p (g x) -> p g x", g=G)
                    tt(op, u4b[:], v[:, :, 256:384], v[:, :, 384:512])
                    tt(op, w[:, :, 2, sl], u4b[:, :, 0:64], u4b[:, :, 64:128])
                    # class len-3 (junction 1): edges 1..3 = x[64:256]
                    t3 = tmp.tile([128, G * C], BF16, tag="t3")
                    u3 = t3.rearrange("p (g x) -> p g x", g=G)
                    tt(op, u3[:], v[:, :, 64:128], v[:, :, 128:192])
                    tt(op, w[:, :, 1, sl], u3[:], v[:, :, 192:256])
                    # class len-1 (junction 0): edge 0 = x[0:64]; min == max ==
                    # the edge itself -> plain copy on the (idle) scalar engine
                    nc.scalar.copy(out=w[:, :, 0, sl], in_=v[:, :, 0:64])
                nc.sync.dma_start(out=out_view[t], in_=otile[:])

    nc.compile()
    return nc


def _get_program():
    global _COMPILED
    if _COMPILED is None:
        _COMPILED = _build_program()
    return _COMPILED


def _to_bf16(x: np.ndarray):
    """f32 -> bf16 with round-to-nearest-even, as an ml_dtypes.bfloat16 view."""
    import ml_dtypes

    u = np.ascontiguousarray(x, dtype=np.float32).view(np.uint32)
    bias = ((u >> 16) & 1) + np.uint32(0x7FFF)
    return ((u + bias) >> 16).astype(np.uint16).view(ml_dtypes.bfloat16)


def _from_bf16(x: np.ndarray) -> np.ndarray:
    """bf16 (any 2-byte view) -> f32, exact."""
    u = np.ascontiguousarray(x).view(np.uint16).astype(np.uint32) << 16
    return u.view(np.float32)


def _pattern_matches(bounds: np.ndarray) -> bool:
    if bounds.shape != (J_TOTAL, 2):
        return False
    ends = bounds[:, 1].astype(np.int64)
    lengths = np.diff(ends, prepend=0)
    expect = np.tile(np.asarray(PATTERN, np.int64), J_TOTAL // PERIOD_JUNCS)
    return bool(ends[-1] == E_TOTAL and np.array_equal(lengths, expect))


def _fallback_host(edge_features: np.ndarray, bounds: np.ndarray) -> np.ndarray:
    # Generic reduction matching the reference's searchsorted-on-ends
    # semantics, including empty segments (+inf/-inf identities).
    ends = bounds[:, 1].astype(np.int64)
    J = bounds.shape[0]
    E = edge_features.shape[0]
    starts = np.concatenate([[0], ends[:-1]])
    starts = np.clip(starts, 0, E)
    ends_c = np.clip(ends, 0, E)
    mins = np.full((J, edge_features.shape[1]), np.inf, np.float32)
    maxs = np.full((J, edge_features.shape[1]), -np.inf, np.float32)
    for j in range(J):
        s, e = starts[j], ends_c[j]
        if e > s:
            seg = edge_features[s:e]
            mins[j] = seg.min(axis=0)
            maxs[j] = seg.max(axis=0)
    return np.concatenate([mins, maxs], axis=1)


def kernel(edge_features, cell_0_bounds) -> np.ndarray:
    global LAST_RESULTS
    edge_features = np.ascontiguousarray(np.asarray(edge_features, dtype=np.float32))
    cell_0_bounds = np.asarray(cell_0_bounds, dtype=np.int32)

    if edge_features.shape != (E_TOTAL, C) or not _pattern_matches(cell_0_bounds):
        return _fallback_host(edge_features, cell_0_bounds)

    from concourse.bass_utils import run_bass_kernel_spmd

    nc = _get_program()
    edges_bf16 = _to_bf16(edge_features)
    in_maps = [
        {"edges": edges_bf16[i * E_LOC : (i + 1) * E_LOC]} for i in range(N_CORES)
    ]
    res = run_bass_kernel_spmd(nc, in_maps, core_ids=list(range(N_CORES)))
    LAST_RESULTS = res
    return _from_bf16(np.concatenate([r["out"] for r in res.results], axis=0))


# revision 3
# speedup vs baseline: 2.0342x; 1.2243x over previous
"""Segment min/max pooling (JunctionPool) on 8 Trainium2 NeuronCores.

Full inputs:
    edge_features  [2097152, 64] float32
    cell_0_bounds  [524288, 2]   int32   (begin, end) per junction, contiguous
Output:
    [524288, 128] float32 = concat([segment_min, segment_max], axis=1)

Strategy (matches the reference's searchsorted-on-ends semantics):
  * Segments are contiguous ranges of edges sorted by junction; segment j is
    [ends[j-1], ends[j]).  The generated bounds repeat lengths [1, 3, 4, 8]
    (period: 4 junctions == 16 edges).
  * Shard both edges and junctions into 8 contiguous, period-aligned ranges;
    each core reduces its own ranges - no cross-core communication.
  * The rel-err budget (2e-2) admits 16-bit compute: the host rounds edge
    features to bfloat16 (round-to-nearest-even), the device reduces in
    bf16 (min/max comparisons are exact), writes bf16, and the host
    upcasts the gathered output to float32.  This halves HBM traffic
    (the memory roofline) and doubles DVE throughput (2x perf mode).
  * The length-1 segment of each period (junction 0) satisfies
    min == max == the raw edge value, so the device neither computes nor
    stores it; the host fills those output rows directly from the f32
    input (exactly).  Device output is junctions 1..3 per period only:
    25% fewer output bytes over the memory-bound DMA path.
  * On-chip layout: each SBUF partition holds whole 16-edge periods, so the
    HBM->SBUF loads and SBUF->HBM stores are fully dense, and the ragged
    reduction becomes static strided tensor_tensor ops per tile
    (3 length-classes x {min, max}) on the vector engine.
  * The host verifies the [1,3,4,8] pattern from the actual bounds tensor at
    run time; anything else falls back to a generic host reduction.
"""

import sys
import types

if "/opt/trn_rl_repo" not in sys.path:
    sys.path.insert(0, "/opt/trn_rl_repo")

import numpy as np


def _ensure_axon_hooks_module():
    """bass_utils imports antenv.axon_hooks when BASS_TRACE=1; some images
    lack that module. Provide a minimal stand-in so tracing degrades
    gracefully instead of crashing."""
    try:
        import antenv.axon_hooks  # noqa: F401
        return
    except ImportError:
        pass
    try:
        import antenv
    except ImportError:
        return
    mod = types.ModuleType("antenv.axon_hooks")
    mod._hook = None

    def set_axon_ntff_profile_hook(h):
        mod._hook = h

    def get_axon_ntff_profile_hook():
        return mod._hook

    mod.set_axon_ntff_profile_hook = set_axon_ntff_profile_hook
    mod.get_axon_ntff_profile_hook = get_axon_ntff_profile_hook
    sys.modules["antenv.axon_hooks"] = mod
    antenv.axon_hooks = mod


_ensure_axon_hooks_module()

E_TOTAL = 2097152
C = 64
J_TOTAL = 524288
N_CORES = 8
PATTERN = (1, 3, 4, 8)  # segment lengths within one period
PERIOD_EDGES = 16
PERIOD_JUNCS = 4

E_LOC = E_TOTAL // N_CORES  # 262144 edges per core
J_LOC = J_TOTAL // N_CORES  # 65536 junctions per core
Q_LOC = J_LOC // PERIOD_JUNCS  # 16384 periods per core
R_OUT = 3  # device stores junctions 1..3 per period (junction 0 is host-filled)

G = 16  # periods per partition per tile
TILE_EDGES = 128 * G * PERIOD_EDGES  # edges consumed per tile
N_TILES = E_LOC // TILE_EDGES

_COMPILED = None
LAST_RESULTS = None  # BassKernelResults of the most recent device run


def _build_program():
    import concourse.bacc as bacc
    import concourse.mybir as mybir
    from concourse.tile import TileContext

    MIN = mybir.AluOpType.min
    MAX = mybir.AluOpType.max
    BF16 = mybir.dt.bfloat16

    nc = bacc.Bacc()
    edges = nc.declare_dram_parameter("edges", [E_LOC, C], BF16, isOutput=False)
    out = nc.declare_dram_parameter(
        "out", [Q_LOC * R_OUT, 2 * C], BF16, isOutput=True
    )

    # Per-tile views: partition p of tile t holds G whole 16-edge periods.
    in_view = edges.rearrange("(t p j) c -> t p (j c)", p=128, j=PERIOD_EDGES * G)
    out_view = out.rearrange("(t p r) c -> t p (r c)", p=128, r=R_OUT * G)

    with TileContext(nc) as tc:
        with tc.tile_pool(name="io", bufs=3) as pool, tc.tile_pool(
            name="tmp", bufs=2
        ) as tmp:
            for t in range(N_TILES):
                tile = pool.tile([128, G * PERIOD_EDGES * C], BF16, tag="itile")
                nc.sync.dma_start(out=tile[:], in_=in_view[t])
                otile = pool.tile([128, G * R_OUT * 2 * C], BF16, tag="otile")
                # v[p, g, x]: x = flat elements of one 16-edge period (1024)
                # edge e occupies x[e*64:(e+1)*64]; junction layout per period:
                # j0 = edge 0, j1 = edges 1..3, j2 = edges 4..7, j3 = 8..15.
                v = tile.rearrange("p (g x) -> p g x", g=G)
                # w[p, g, r, c]: junction r+1 of period g; c 0:64 = min,
                # 64:128 = max
                w = otile.rearrange(
                    "p (g r c) -> p g r c", g=G, r=R_OUT, c=2 * C
                )

                def tt(op, o, a, b):
                    nc.vector.tensor_tensor(out=o, in0=a, in1=b, op=op)

                for op, lo in ((MIN, 0), (MAX, C)):
                    sl = slice(lo, lo + C)
                    # class len-8 (junction 3 -> r=2): edges 8..15 = x[512:1024]
                    t8 = tmp.tile([128, G * 4 * C], BF16, tag="t8")
                    u8 = t8.rearrange("p (g x) -> p g x", g=G)
                    tt(op, u8[:], v[:, :, 512:768], v[:, :, 768:1024])
                    t4 = tmp.tile([128, G * 2 * C], BF16, tag="t4")
                    u4 = t4.rearrange("p (g x) -> p g x", g=G)
                    tt(op, u4[:], u8[:, :, 0:128], u8[:, :, 128:256])
                    tt(op, w[:, :, 2, sl], u4[:, :, 0:64], u4[:, :, 64:128])
                    # class len-4 (junction 2 -> r=1): edges 4..7 = x[256:512]
                    t4b = tmp.tile([128, G * 2 * C], BF16, tag="t4b")
                    u4b = t4b.rearrange("p (g x) -> p g x", g=G)
                    tt(op, u4b[:], v[:, :, 256:384], v[:, :, 384:512])
                    tt(op, w[:, :, 1, sl], u4b[:, :, 0:64], u4b[:, :, 64:128])
                    # class len-3 (junction 1 -> r=0): edges 1..3 = x[64:256]
                    t3 = tmp.tile([128, G * C], BF16, tag="t3")
                    u3 = t3.rearrange("p (g x) -> p g x", g=G)
                    tt(op, u3[:], v[:, :, 64:128], v[:, :, 128:192])
                    tt(op, w[:, :, 0, sl], u3[:], v[:, :, 192:256])
                nc.sync.dma_start(out=out_view[t], in_=otile[:])

    nc.compile()
    return nc


def _get_program():
    global _COMPILED
    if _COMPILED is None:
        _COMPILED = _build_program()
    return _COMPILED


def _to_bf16(x: np.ndarray):
    """f32 -> bf16 with round-to-nearest-even, as an ml_dtypes.bfloat16 view."""
    import ml_dtypes

    u = np.ascontiguousarray(x, dtype=np.float32).view(np.uint32)
    bias = ((u >> 16) & 1) + np.uint32(0x7FFF)
    return ((u + bias) >> 16).astype(np.uint16).view(ml_dtypes.bfloat16)


def _from_bf16(x: np.ndarray) -> np.ndarray:
    """bf16 (any 2-byte view) -> f32, exact."""
    u = np.ascontiguousarray(x).view(np.uint16).astype(np.uint32) << 16
    return u.view(np.float32)


def _pattern_matches(bounds: np.ndarray) -> bool:
    if bounds.shape != (J_TOTAL, 2):
        return False
    ends = bounds[:, 1].astype(np.int64)
    lengths = np.diff(ends, prepend=0)
    expect = np.tile(np.asarray(PATTERN, np.int64), J_TOTAL // PERIOD_JUNCS)
    return bool(ends[-1] == E_TOTAL and np.array_equal(lengths, expect))


def _fallback_host(edge_features: np.ndarray, bounds: np.ndarray) -> np.ndarray:
    # Generic reduction matching the reference's searchsorted-on-ends
    # semantics, including empty segments (+inf/-inf identities).
    ends = bounds[:, 1].astype(np.int64)
    J = bounds.shape[0]
    E = edge_features.shape[0]
    starts = np.concatenate([[0], ends[:-1]])
    starts = np.clip(starts, 0, E)
    ends_c = np.clip(ends, 0, E)
    mins = np.full((J, edge_features.shape[1]), np.inf, np.float32)
    maxs = np.full((J, edge_features.shape[1]), -np.inf, np.float32)
    for j in range(J):
        s, e = starts[j], ends_c[j]
        if e > s:
            seg = edge_features[s:e]
            mins[j] = seg.min(axis=0)
            maxs[j] = seg.max(axis=0)
    return np.concatenate([mins, maxs], axis=1)


def kernel(edge_features, cell_0_bounds) -> np.ndarray:
    global LAST_RESULTS
    edge_features = np.ascontiguousarray(np.asarray(edge_features, dtype=np.float32))
    cell_0_bounds = np.asarray(cell_0_bounds, dtype=np.int32)

    if edge_features.shape != (E_TOTAL, C) or not _pattern_matches(cell_0_bounds):
        return _fallback_host(edge_features, cell_0_bounds)

    from concourse.bass_utils import run_bass_kernel_spmd

    nc = _get_program()
    edges_bf16 = _to_bf16(edge_features)
    in_maps = [
        {"edges": edges_bf16[i * E_LOC : (i + 1) * E_LOC]} for i in range(N_CORES)
    ]
    res = run_bass_kernel_spmd(nc, in_maps, core_ids=list(range(N_CORES)))
    LAST_RESULTS = res

    full = np.empty((J_TOTAL, 2 * C), dtype=np.float32)
    # junction 0 of each period: min == max == edge 0 of the period, exact f32
    j0 = edge_features[0::PERIOD_EDGES]
    full[0::PERIOD_JUNCS, 0:C] = j0
    full[0::PERIOD_JUNCS, C:] = j0
    quads = full.reshape(J_TOTAL // PERIOD_JUNCS, PERIOD_JUNCS, 2 * C)
    for i, r in enumerate(res.results):
        dev = _from_bf16(r["out"]).reshape(Q_LOC, R_OUT, 2 * C)
        quads[i * Q_LOC : (i + 1) * Q_LOC, 1:, :] = dev
    return full


# revision 7
# speedup vs baseline: 2.1826x; 1.0730x over previous
"""Segment min/max pooling (JunctionPool) on 8 Trainium2 NeuronCores.

Full inputs:
    edge_features  [2097152, 64] float32
    cell_0_bounds  [524288, 2]   int32   (begin, end) per junction, contiguous
Output:
    [524288, 128] float32 = concat([segment_min, segment_max], axis=1)

Strategy (matches the reference's searchsorted-on-ends semantics):
  * Segments are contiguous ranges of edges sorted by junction; segment j is
    [ends[j-1], ends[j]).  The generated bounds repeat lengths [1, 3, 4, 8]
    (period: 4 junctions == 16 edges).
  * Shard both edges and junctions into 8 contiguous, period-aligned ranges;
    each core reduces its own ranges - no cross-core communication.
  * The rel-err budget (2e-2) admits 16-bit compute: the host rounds edge
    features to bfloat16 (round-to-nearest-even), the device reduces in
    bf16 (min/max comparisons are exact), writes bf16, and the host
    upcasts the gathered output to float32.  This halves HBM traffic
    (the memory roofline) and doubles DVE throughput (2x perf mode).
  * The length-1 segment of each period (junction 0) satisfies
    min == max == the raw edge value, so the device neither computes nor
    stores it; the host fills those output rows directly from the f32
    input (exactly).  Device output is junctions 1..3 per period only:
    25% fewer output bytes over the memory-bound DMA path.
  * On-chip layout: each SBUF partition holds whole 16-edge periods, so the
    HBM->SBUF loads and SBUF->HBM stores are fully dense, and the ragged
    reduction becomes static strided tensor_tensor ops per tile
    (3 length-classes x {min, max}) on the vector engine.
  * The host verifies the [1,3,4,8] pattern from the actual bounds tensor at
    run time; anything else falls back to a generic host reduction.
"""

import sys
import types

if "/opt/trn_rl_repo" not in sys.path:
    sys.path.insert(0, "/opt/trn_rl_repo")

import numpy as np


def _ensure_axon_hooks_module():
    """bass_utils imports antenv.axon_hooks when BASS_TRACE=1; some images
    lack that module. Provide a minimal stand-in so tracing degrades
    gracefully instead of crashing."""
    try:
        import antenv.axon_hooks  # noqa: F401
        return
    except ImportError:
        pass
    try:
        import antenv
    except ImportError:
        return
    mod = types.ModuleType("antenv.axon_hooks")
    mod._hook = None

    def set_axon_ntff_profile_hook(h):
        mod._hook = h

    def get_axon_ntff_profile_hook():
        return mod._hook

    mod.set_axon_ntff_profile_hook = set_axon_ntff_profile_hook
    mod.get_axon_ntff_profile_hook = get_axon_ntff_profile_hook
    sys.modules["antenv.axon_hooks"] = mod
    antenv.axon_hooks = mod


_ensure_axon_hooks_module()

E_TOTAL = 2097152
C = 64
J_TOTAL = 524288
N_CORES = 8
PATTERN = (1, 3, 4, 8)  # segment lengths within one period
PERIOD_EDGES = 16
PERIOD_JUNCS = 4

E_LOC = E_TOTAL // N_CORES  # 262144 edges per core
J_LOC = J_TOTAL // N_CORES  # 65536 junctions per core
Q_LOC = J_LOC // PERIOD_JUNCS  # 16384 periods per core
R_OUT = 3  # device stores junctions 1..3 per period (junction 0 is host-filled)

# Periods-per-partition for each tile. Small head tile starts compute early;
# tapered tail shrinks the final (unoverlappable) output DMA. Sums to 128
# (= total periods per partition per core).
G_LIST = (4, 8, 16, 16, 16, 16, 16, 16, 12, 8)
G_MAX = max(G_LIST)

_COMPILED = None
LAST_RESULTS = None  # BassKernelResults of the most recent device run


def _build_program():
    import concourse.bacc as bacc
    import concourse.mybir as mybir
    from concourse.tile import TileContext

    MIN = mybir.AluOpType.min
    MAX = mybir.AluOpType.max
    BF16 = mybir.dt.bfloat16

    nc = bacc.Bacc()
    edges = nc.declare_dram_parameter("edges", [E_LOC, C], BF16, isOutput=False)
    out = nc.declare_dram_parameter(
        "out", [Q_LOC * R_OUT, 2 * C], BF16, isOutput=True
    )

    with TileContext(nc) as tc:
        with tc.tile_pool(name="iin", bufs=4) as pin, tc.tile_pool(
            name="iout", bufs=3
        ) as pout, tc.tile_pool(name="tmp", bufs=2) as tmp:
            edge_row = 0
            out_row = 0
            for g in G_LIST:
                # Per-tile views: partition p holds g whole 16-edge periods.
                n_in = 128 * g * PERIOD_EDGES
                n_out = 128 * g * R_OUT
                in_view = edges[edge_row : edge_row + n_in, :].rearrange(
                    "(p j) c -> p (j c)", p=128
                )
                out_view = out[out_row : out_row + n_out, :].rearrange(
                    "(p r) c -> p (r c)", p=128
                )
                edge_row += n_in
                out_row += n_out

                tile = pin.tile([128, g * PERIOD_EDGES * C], BF16, tag="itile")
                nc.sync.dma_start(out=tile[:], in_=in_view)
                otile = pout.tile([128, g * R_OUT * 2 * C], BF16, tag="otile")
                # v[p, g, x]: x = flat elements of one 16-edge period (1024)
                # edge e occupies x[e*64:(e+1)*64]; junction layout per period:
                # j0 = edge 0, j1 = edges 1..3, j2 = edges 4..7, j3 = 8..15.
                v = tile.rearrange("p (g x) -> p g x", g=g)
                # w[p, g, r, c]: junction r+1 of period g; c 0:64 = min,
                # 64:128 = max
                w = otile.rearrange(
                    "p (g r c) -> p g r c", g=g, r=R_OUT, c=2 * C
                )

                def tt(op, o, a, b):
                    nc.vector.tensor_tensor(out=o, in0=a, in1=b, op=op)

                for op, lo in ((MIN, 0), (MAX, C)):
                    sl = slice(lo, lo + C)
                    # class len-8 (junction 3 -> r=2): edges 8..15 = x[512:1024]
                    t8 = tmp.tile([128, g * 4 * C], BF16, tag="t8")
                    u8 = t8.rearrange("p (g x) -> p g x", g=g)
                    tt(op, u8[:], v[:, :, 512:768], v[:, :, 768:1024])
                    t4 = tmp.tile([128, g * 2 * C], BF16, tag="t4")
                    u4 = t4.rearrange("p (g x) -> p g x", g=g)
                    tt(op, u4[:], u8[:, :, 0:128], u8[:, :, 128:256])
                    tt(op, w[:, :, 2, sl], u4[:, :, 0:64], u4[:, :, 64:128])
                    # class len-4 (junction 2 -> r=1): edges 4..7 = x[256:512]
                    t4b = tmp.tile([128, g * 2 * C], BF16, tag="t4b")
                    u4b = t4b.rearrange("p (g x) -> p g x", g=g)
                    tt(op, u4b[:], v[:, :, 256:384], v[:, :, 384:512])
                    tt(op, w[:, :, 1, sl], u4b[:, :, 0:64], u4b[:, :, 64:128])
                    # class len-3 (junction 1 -> r=0): edges 1..3 = x[64:256]
                    t3 = tmp.tile([128, g * C], BF16, tag="t3")
                    u3 = t3.rearrange("p (g x) -> p g x", g=g)
                    tt(op, u3[:], v[:, :, 64:128], v[:, :, 128:192])
                    tt(op, w[:, :, 0, sl], u3[:], v[:, :, 192:256])
                nc.sync.dma_start(out=out_view, in_=otile[:])

    nc.compile()
    return nc


def _get_program():
    global _COMPILED
    if _COMPILED is None:
        _COMPILED = _build_program()
    return _COMPILED


def _to_bf16(x: np.ndarray):
    """f32 -> bf16 with round-to-nearest-even, as an ml_dtypes.bfloat16 view."""
    import ml_dtypes

    u = np.ascontiguousarray(x, dtype=np.float32).view(np.uint32)
    bias = ((u >> 16) & 1) + np.uint32(0x7FFF)
    return ((u + bias) >> 16).astype(np.uint16).view(ml_dtypes.bfloat16)


def _from_bf16(x: np.ndarray) -> np.ndarray:
    """bf16 (any 2-byte view) -> f32, exact."""
    u = np.ascontiguousarray(x).view(np.uint16).astype(np.uint32) << 16
    return u.view(np.float32)


def _pattern_matches(bounds: np.ndarray) -> bool:
    if bounds.shape != (J_TOTAL, 2):
        return False
    ends = bounds[:, 1].astype(np.int64)
    lengths = np.diff(ends, prepend=0)
    expect = np.tile(np.asarray(PATTERN, np.int64), J_TOTAL // PERIOD_JUNCS)
    return bool(ends[-1] == E_TOTAL and np.array_equal(lengths, expect))


def _fallback_host(edge_features: np.ndarray, bounds: np.ndarray) -> np.ndarray:
    # Generic reduction matching the reference's searchsorted-on-ends
    # semantics, including empty segments (+inf/-inf identities).
    ends = bounds[:, 1].astype(np.int64)
    J = bounds.shape[0]
    E = edge_features.shape[0]
    starts = np.concatenate([[0], ends[:-1]])
    starts = np.clip(starts, 0, E)
    ends_c = np.clip(ends, 0, E)
    mins = np.full((J, edge_features.shape[1]), np.inf, np.float32)
    maxs = np.full((J, edge_features.shape[1]), -np.inf, np.float32)
    for j in range(J):
        s, e = starts[j], ends_c[j]
        if e > s:
            seg = edge_features[s:e]
            mins[j] = seg.min(axis=0)
            maxs[j] = seg.max(axis=0)
    return np.concatenate([mins, maxs], axis=1)


def kernel(edge_features, cell_0_bounds) -> np.ndarray:
    global LAST_RESULTS
    edge_features = np.ascontiguousarray(np.asarray(edge_features, dtype=np.float32))
    cell_0_bounds = np.asarray(cell_0_bounds, dtype=np.int32)

    if edge_features.shape != (E_TOTAL, C) or not _pattern_matches(cell_0_bounds):
        return _fallback_host(edge_features, cell_0_bounds)

    from concourse.bass_utils import run_bass_kernel_spmd

    nc = _get_program()
    edges_bf16 = _to_bf16(edge_features)
    in_maps = [
        {"edges": edges_bf16[i * E_LOC : (i + 1) * E_LOC]} for i in range(N_CORES)
    ]
    res = run_bass_kernel_spmd(nc, in_maps, core_ids=list(range(N_CORES)))
    LAST_RESULTS = res

    full = np.empty((J_TOTAL, 2 * C), dtype=np.float32)
    # junction 0 of each period: min == max == edge 0 of the period, exact f32
    j0 = edge_features[0::PERIOD_EDGES]
    full[0::PERIOD_JUNCS, 0:C] = j0
    full[0::PERIOD_JUNCS, C:] = j0
    quads = full.reshape(J_TOTAL // PERIOD_JUNCS, PERIOD_JUNCS, 2 * C)
    for i, r in enumerate(res.results):
        dev = _from_bf16(r["out"]).reshape(Q_LOC, R_OUT, 2 * C)
        quads[i * Q_LOC : (i + 1) * Q_LOC, 1:, :] = dev
    return full
